# revision 1
# baseline (speedup 1.0000x reference)
"""Trainium2 Bass kernel for nn_Encoder_Postnet (length-regulator gather + per-frame linears).

Contract: kernel(**inputs) takes FULL numpy inputs (as produced by
setup_inputs) and returns the FULL [B, T, H] float32 output. Internally the
batch dim is sharded across 8 NeuronCores (pure data parallel, 4 batches per
core); the tiny Linear(1,H) params are replicated.

Per-core algorithm (BPC=4 batches, T=4096 frames, P=512 phonemes, H=512),
pipelined in 8 stages of 4 x 128-frame chunks so gathers start early:
  1. idx[b,t] = cumsum_t(align[b,t] != align[b,t-1])  -- DVE compare + scan
  2. PE-transpose idx chunks into per-partition layout -> gather offsets
  3. per-chunk indirect-DMA gathers of enc rows (bf16) from HBM
     (HW consumes exactly one offset per dest partition per call)
  4. K=11 bf16 PE matmul per chunk for the rank-1 updates, with hi/lo
     weight splits so pos*w_pos keeps ~fp32 accuracy (fp32 matmul is 2
     slow passes on TRN2, bf16 is 1 fast pass)
  5. one DVE add (gathered + psum) per chunk, f32 writes on alternating
     HWDGE rings (sync/scalar)
"""

import sys

if "/opt/trn_rl_repo" not in sys.path:
    sys.path.insert(0, "/opt/trn_rl_repo")

from contextlib import ExitStack

import numpy as np

import concourse.bass as bass
import concourse.tile as tile
from concourse import bacc, mybir
from concourse.bass_utils import run_bass_kernel_spmd
from concourse.masks import make_identity

B, T, P, H = 32, 4096, 512, 512
NCORES = 8
BPC = B // NCORES            # batches per core
TILE_T = 128                 # frames per tile (partition dim)
NCHUNK = T // TILE_T         # 32 tiles per batch
K_MM = 11                    # bf16 matmul contraction (hi/lo split, see below)
F32 = mybir.dt.float32
BF16 = mybir.dt.bfloat16
I32 = mybir.dt.int32
ADD = mybir.AluOpType.add
SUB = mybir.AluOpType.subtract
NE = mybir.AluOpType.not_equal


def _emit(ctx: ExitStack, tc: tile.TileContext, enc, pitch_bf, beats_bf,
          align, wmat, aux, out):
    nc = tc.nc
    const = ctx.enter_context(tc.tile_pool(name="const", bufs=1))
    apool = ctx.enter_context(tc.tile_pool(name="apool", bufs=1))
    gpool = ctx.enter_context(tc.tile_pool(name="gpool", bufs=24))
    opool = ctx.enter_context(tc.tile_pool(name="opool", bufs=20))
    ppool = ctx.enter_context(tc.tile_pool(name="ppool", bufs=7, space="PSUM"))
    tpsum = ctx.enter_context(tc.tile_pool(name="tpsum", bufs=1, space="PSUM"))

    # --- idx chain first: everything else waits on nothing, but the first
    # gather waits on align -> change -> scan -> transpose -> offsets
    align_sb = const.tile([BPC, T], I32)
    nc.sync.dma_start(align_sb[:], align[:])
    change = const.tile([BPC, T], F32)
    nc.vector.memset(change[:, 0:1], 0.0)
    zeros = const.tile([BPC, T], F32)
    idxf = const.tile([BPC, T], F32)
    ident = const.tile([BPC, BPC], F32)
    make_identity(nc, ident[:])
    idxT_ps = tpsum.tile([TILE_T, NCHUNK * BPC], F32)
    offs = [const.tile([TILE_T, NCHUNK], I32, tag=f"offs{b}",
                       name=f"offs{b}")
            for b in range(BPC)]
    # [128, BPC, NCHUNK] view of the PSUM transposes; converts read PSUM
    idxT3 = idxT_ps[:].rearrange("p (c b) -> p b c", b=BPC)

    # variable stage sizes (in chunks): tiny first stages so the first
    # gathers start after a ~1-chunk scan chain instead of the full setup
    STAGES = [1, 1, 2, 4, 8, 8, 8]
    SB = [0]
    for n_ in STAGES:
        SB.append(SB[-1] + n_)
    NSTAGE = len(STAGES)

    def scan_piece_a(st):
        lo, hi = SB[st] * TILE_T, SB[st + 1] * TILE_T
        s0 = max(lo, 1)
        nc.vector.memset(zeros[:, lo:hi], 0.0)
        nc.vector.tensor_tensor(change[:, s0:hi], align_sb[:, s0:hi],
                                align_sb[:, s0 - 1:hi - 1], op=NE)
        carry = 0.0 if st == 0 else idxf[:, lo - 1:lo]
        nc.vector.tensor_tensor_scan(idxf[:, lo:hi], change[:, lo:hi],
                                     zeros[:, lo:hi], carry,
                                     op0=ADD, op1=ADD)

    def scan_piece_b(st):
        for c in range(SB[st], SB[st + 1]):
            nc.tensor.transpose(idxT_ps[:, c * BPC:(c + 1) * BPC],
                                idxf[:, c * TILE_T:(c + 1) * TILE_T],
                                ident[:])

    def scan_piece_c(st):
        for b in range(BPC):
            nc.vector.tensor_scalar_add(
                offs[b][:, SB[st]:SB[st + 1]],
                idxT3[:, b, SB[st]:SB[st + 1]], float(b * P))

    def emit_scan_stage(st):
        scan_piece_a(st)
        scan_piece_b(st)
        scan_piece_c(st)

    emit_scan_stage(0)

    # --- W [11, H] bf16, assembled on the host (hi/lo split of w_pos/
    # w_pitch/w_beats + bf16 biases) and loaded with one DMA. fp32 matmul
    # lowers to two ~1us passes on TRN2, so the rank-update runs in bf16:
    #   pos*w_pos = (t_hi + t_lo) * (w_hi + w_lo),  t_hi = 16*(t//16), exact
    # W rows: [wpos_hi, wpos_lo, wpos_hi, wpos_lo, wpit_hi, wpit_lo,
    #          wbea_hi, wbea_lo, b_pitch, b_beats, b_pos]
    # A rows: [t_hi, t_hi, t_lo, t_lo, pitch, pitch, beats, beats, 1, 1, 1]
    W = const.tile([K_MM, H], BF16)
    nc.sync.dma_start(W[:], wmat[:])

    # --- A tiles, persistent per batch: [t_hi, t_hi, t_lo, t_lo, pitch,
    # pitch, beats, beats, 1, 1, 1]; t_hi/t_lo/ones from host aux and
    # pitch/beats pre-cast to bf16 on the host (exact-layout marshaling)
    As = []
    for b in range(BPC):
        A = apool.tile([K_MM, T], BF16, tag=f"A{b}")
        nc.sync.dma_start(A[0:4, :], aux[0:4, :])
        nc.sync.dma_start(A[4:5, :], pitch_bf[b:b + 1, :])
        nc.sync.dma_start(A[5:6, :], pitch_bf[b:b + 1, :])
        nc.sync.dma_start(A[6:7, :], beats_bf[b:b + 1, :])
        nc.sync.dma_start(A[7:8, :], beats_bf[b:b + 1, :])
        nc.sync.dma_start(A[8:11, :], aux[4:7, :])
        As.append(A)

    for st in range(NSTAGE):
        # spread the NEXT stage's scan chain through this stage's main loop
        # so the DVE interleaves it with the adds instead of blocking them
        stage_chunks = [(b, c) for b in range(BPC)
                        for c in range(SB[st], SB[st + 1])]
        for i, (b, c) in enumerate(stage_chunks):
            n = len(stage_chunks)
            if st + 1 < NSTAGE:
                if i == max(1, n // 4):
                    scan_piece_a(st + 1)
                elif i == max(2, n // 2):
                    scan_piece_b(st + 1)
                elif i == max(3, 3 * n // 4):
                    scan_piece_c(st + 1)
            # HW indirect DMA consumes exactly one offset per dest
            # partition: per-chunk gathers, 128 descriptors x one H-row
            gt = gpool.tile([TILE_T, H], BF16)
            nc.gpsimd.indirect_dma_start(
                out=gt[:],
                out_offset=None,
                in_=enc[:],
                in_offset=bass.IndirectOffsetOnAxis(
                    ap=offs[b][:, c:c + 1], axis=0),
            )
            ps = ppool.tile([TILE_T, H], F32)
            nc.tensor.matmul(ps[:],
                             lhsT=As[b][:, c * TILE_T:(c + 1) * TILE_T],
                             rhs=W[:], start=True, stop=True)
            ot = opool.tile([TILE_T, H], F32)
            nc.vector.tensor_tensor(ot[:], gt[:], ps[:], op=ADD)
            # alternate the two HWDGE rings (SP via sync, ACT via scalar)
            weng = nc.sync if c % 2 == 0 else nc.scalar
            weng.dma_start(
                out[b * T + c * TILE_T: b * T + (c + 1) * TILE_T, :],
                ot[:])


_CACHED = None


def _build():
    global _CACHED
    if _CACHED is not None:
        return _CACHED
    nc = bacc.Bacc("TRN2", target_bir_lowering=False, debug=False,
                   num_swdge_queues=2)
    enc = nc.dram_tensor("enc", (BPC * P, H), BF16,
                     kind="ExternalInput").ap()
    pitch_bf = nc.dram_tensor("pitch_bf", (BPC, T), BF16,
                              kind="ExternalInput").ap()
    beats_bf = nc.dram_tensor("beats_bf", (BPC, T), BF16,
                              kind="ExternalInput").ap()
    align = nc.dram_tensor("align", (BPC, T), I32, kind="ExternalInput").ap()
    wmat = nc.dram_tensor("wmat", (K_MM, H), BF16, kind="ExternalInput").ap()
    aux = nc.dram_tensor("aux", (7, T), BF16, kind="ExternalInput").ap()
    out = nc.dram_tensor("out", (BPC * T, H), F32, kind="ExternalOutput").ap()

    with tile.TileContext(nc) as tc:
        with ExitStack() as ctx:
            _emit(ctx, tc, enc, pitch_bf, beats_bf, align, wmat, aux,
                  out)
    nc.compile()
    _CACHED = nc
    return nc


def make_in_maps(encoder_out, pitch, beats, align_phone,
                 w_pitch, b_pitch, w_beats, b_beats, w_pos, b_pos):
    import ml_dtypes
    bf16 = ml_dtypes.bfloat16
    t = np.arange(T, dtype=np.float32)
    t_hi = np.float32(16.0) * np.floor(t / 16.0).astype(np.float32)
    t_lo = t - t_hi
    ones = np.ones(T, np.float32)
    aux = np.stack([t_hi, t_hi, t_lo, t_lo, ones, ones, ones]).astype(bf16)

    def hilo(w):
        w = np.asarray(w, np.float32)
        hi = w.astype(bf16)
        lo = (w - hi.astype(np.float32)).astype(bf16)
        return hi, lo

    wpos_hi, wpos_lo = hilo(w_pos)
    wpit_hi, wpit_lo = hilo(w_pitch)
    wbea_hi, wbea_lo = hilo(w_beats)
    wmat = np.stack([wpos_hi, wpos_lo, wpos_hi, wpos_lo, wpit_hi, wpit_lo,
                     wbea_hi, wbea_lo,
                     np.asarray(b_pitch, np.float32).astype(bf16),
                     np.asarray(b_beats, np.float32).astype(bf16),
                     np.asarray(b_pos, np.float32).astype(bf16)])
    reps = {
        "aux": aux,
        "wmat": wmat,
    }
    in_maps = []
    for r in range(NCORES):
        s = slice(r * BPC, (r + 1) * BPC)
        in_maps.append({
            "enc": np.ascontiguousarray(
                encoder_out[s], np.float32).reshape(BPC * P, H).astype(
                    ml_dtypes.bfloat16),
            "pitch_bf": np.ascontiguousarray(pitch[s]).astype(
                ml_dtypes.bfloat16),
            "beats_bf": np.ascontiguousarray(beats[s]).astype(
                ml_dtypes.bfloat16),
            "align": np.ascontiguousarray(align_phone[s], np.int32),
            **reps,
        })
    return in_maps


def _run_in_subprocess(kwargs):
    """Fallback for a wedged in-process PJRT client: re-run this module in a
    fresh interpreter (fresh device boot), passing inputs via pickle."""
    import os
    import pickle
    import subprocess
    import tempfile

    with tempfile.TemporaryDirectory() as td:
        inp = os.path.join(td, "in.pkl")
        outp = os.path.join(td, "out.npy")
        with open(inp, "wb") as f:
            pickle.dump(kwargs, f)
        code = (
            "import pickle, numpy as np, importlib.util\n"
            f"spec = importlib.util.spec_from_file_location('k', {__file__!r})\n"
            "m = importlib.util.module_from_spec(spec)\n"
            "spec.loader.exec_module(m)\n"
            f"ins = pickle.load(open({inp!r}, 'rb'))\n"
            f"np.save({outp!r}, m.kernel(**ins, _no_fallback=True))\n"
        )
        subprocess.run([sys.executable, "-c", code], check=True, timeout=1700)
        return np.load(outp)


def kernel(encoder_out, pitch, beats, w_pitch, b_pitch, w_beats, b_beats,
           w_pos, b_pos, align_phone, _trace=False, _no_fallback=False):
    kwargs = dict(encoder_out=np.asarray(encoder_out),
                  pitch=np.asarray(pitch), beats=np.asarray(beats),
                  w_pitch=np.asarray(w_pitch), b_pitch=np.asarray(b_pitch),
                  w_beats=np.asarray(w_beats), b_beats=np.asarray(b_beats),
                  w_pos=np.asarray(w_pos), b_pos=np.asarray(b_pos),
                  align_phone=np.asarray(align_phone))
    nc = _build()
    in_maps = make_in_maps(encoder_out, pitch, beats, align_phone,
                           w_pitch, b_pitch, w_beats, b_beats, w_pos, b_pos)

    def attempt():
        # materialize eagerly so device failures surface inside the guard
        res = run_bass_kernel_spmd(nc, in_maps, core_ids=list(range(NCORES)),
                                   trace=_trace)
        return res, np.concatenate(
            [np.asarray(res.results[r]["out"]).reshape(BPC, T, H)
             for r in range(NCORES)], axis=0)

    import time
    res = out = None
    for i in range(2):
        try:
            res, out = attempt()
            break
        except Exception:
            # rare flaky device hang (NRT_EXEC_UNIT_UNRECOVERABLE)
            time.sleep(5.0)
    if out is None:
        if _no_fallback:
            res, out = attempt()
        else:
            # fresh interpreter = fresh PJRT client + device reset
            try:
                return _run_in_subprocess(kwargs)
            except Exception:
                time.sleep(10.0)
                return _run_in_subprocess(kwargs)
    if _trace:
        kernel.last_results = res
    return out

